# revision 3
# baseline (speedup 1.0000x reference)
"""ChannelAtten v2: fp8-DoubleRow dw-conv, SBUF-resident v, fused proj.

Per core (batch, H-half shard): qkv 1x1 conv with b_qkv folded via a ones
channel (q,k channels via one fp8 DoubleRow matmul each, K interleaved 96x2;
v channels bf16, K split 128+65). Depthwise 3x3 via windowed-AP diagonal
matmuls on ring buffers: q,k fully on PE in fp8 ((kh0+kh2) row-pair DR +
kh1 dummy-DR, b_dw added during the ACT psum->sbuf copy), v via PE bf16 kh1
+ ACT-init/DVE/Pool scalar chains merged straight into SBUF-resident v.
Gram + self-gram (per-channel sumsq) accumulate in PSUM across the row loop;
one [48,200] AllReduce carries gram+sumsq; softmax on-chip; proj is folded
into the attention matrix (PT = blockdiag(attn) @ wprojT) so phase C is one
GEMM streaming v from SBUF, with b_proj added during the output copy.
"""

import sys

sys.path.insert(0, "/opt/trn_rl_repo")

import numpy as np
import ml_dtypes

import concourse.bass as bass
import concourse.mybir as mybir
import concourse.tile as tile
from concourse import bacc
from concourse.bass_utils import run_bass_kernel_spmd

F32 = mybir.dt.float32
BF16 = mybir.dt.bfloat16
FP8 = mybir.dt.float8e4
AF = mybir.ActivationFunctionType
OP = mybir.AluOpType
AX = mybir.AxisListType
DR = mybir.MatmulPerfMode.DoubleRow

DIM = 192
HEAD_DIM = 48
NH = 4
H = 256
W = 256
B = 4
N_CORES = 8
HALF = H // 2          # 128 output rows per shard
PR = HALF + 2          # 130 padded rows per shard
SCALE = HEAD_DIM ** -0.5
EPS = 1e-12
RING = 16              # qk ring rows (fp8)
RINGV = 8              # v ring rows (bf16)
NITER = PR // 2        # 65 qkv row-pair iterations
NDW = NITER - 1        # 64 output row-pair iterations

QK_T = [(0, 128), (128, 128), (256, 128)]   # qkv channels 0:384
V_T = [(0, 128), (128, 64)]                 # offsets within v block 384:576

_CACHED = {}


def _colwin(kw):
    # (in-col slice, out-col slice) for a kw tap — exact zero-pad semantics
    if kw == 0:
        return slice(0, 255), slice(1, 256)
    if kw == 1:
        return slice(0, 256), slice(0, 256)
    return slice(1, 256), slice(0, 255)


def _build_nc(repeat=1, no_cc=False, dbg=False):
    nc = bacc.Bacc("TRN2", target_bir_lowering=False, debug=False,
                   enable_asserts=True, num_devices=N_CORES)

    x8_d = nc.dram_tensor("x8", [97, 2, PR, W], FP8, kind="ExternalInput").ap()
    xbf_d = nc.dram_tensor("xbf", [DIM + 1, PR, W], BF16,
                           kind="ExternalInput").ap()
    wq8_d = nc.dram_tensor("wq8", [97, 2, 384], FP8, kind="ExternalInput").ap()
    wqv_d = nc.dram_tensor("wqv", [DIM + 1, DIM], BF16,
                           kind="ExternalInput").ap()
    # qk dw diag units: [128, (3 tiles x 12 units) x 2 x 128] fp8
    # units 0-2: (kh0,kh2) pair per kw; 3-5: kh1 dummy per kw;
    # 6-8: kh0-only dummy per kw; 9-11: kh2-only dummy per kw
    wdw8_d = nc.dram_tensor("wdw8", [128, 3 * 12 * 2 * 128], FP8,
                            kind="ExternalInput").ap()
    # v dw diag bf16: kh1 [2 tiles x 3 kw] then kh2 [2 tiles x 3 kw]
    wdwv_d = nc.dram_tensor("wdwv", [128, 2 * 2 * 3 * 128], BF16,
                            kind="ExternalInput").ap()
    # scalar columns f32: per v tile 7 (6 chain taps + b_dw), then qk b_dw x3,
    # then b_proj (2 cols: ch 0:128, 128:192)
    wvs_d = nc.dram_tensor("wvs", [128, 19], F32, kind="ExternalInput").ap()
    wproj_d = nc.dram_tensor("wprojT", [DIM, DIM], BF16,
                             kind="ExternalInput").ap()
    ident_d = nc.dram_tensor("ident", [128, 128], F32, kind="ExternalInput").ap()
    out_d = nc.dram_tensor("out_sh", [DIM, HALF, W], F32,
                           kind="ExternalOutput").ap()
    if dbg:
        dbg1_d = nc.dram_tensor("dbg1", [48, 592], F32,
                                kind="ExternalOutput").ap()
        dbg2_d = nc.dram_tensor("dbg2", [128, 384], F32,
                                kind="ExternalOutput").ap()

    with tile.TileContext(nc) as tc:
        for _rep in range(repeat):
            with (
                tc.tile_pool(name="const", bufs=1) as constp,
                tc.tile_pool(name="xin", bufs=2) as xpool,
                tc.tile_pool(name="dq", bufs=2) as dqpool,
                tc.tile_pool(name="acc", bufs=2) as accp,
                tc.tile_pool(name="sT", bufs=3) as sTpool,
                tc.tile_pool(name="small", bufs=1) as smallp,
                tc.tile_pool(name="outsb", bufs=2) as outsbp,
                tc.tile_pool(name="dram", bufs=1, space="DRAM") as dram,
            ):
                # ---- constants ----
                wq8 = constp.tile([97, 2, 384], FP8)
                nc.sync.dma_start(wq8[:], wq8_d[:])
                wqv_a = constp.tile([128, DIM], BF16)
                wqv_b = constp.tile([65, DIM], BF16)
                nc.sync.dma_start(wqv_a[:], wqv_d[0:128, :])
                nc.sync.dma_start(wqv_b[:], wqv_d[128:193, :])
                wdw8 = constp.tile([128, 3 * 12 * 2 * 128], FP8)
                nc.sync.dma_start(wdw8[:], wdw8_d[:])
                wdwv = constp.tile([128, 2 * 2 * 3 * 128], BF16)
                nc.sync.dma_start(wdwv[:], wdwv_d[:])
                wvs = constp.tile([128, 19], F32)
                nc.sync.dma_start(wvs[:], wvs_d[:])
                wp_a = constp.tile([128, DIM], BF16)
                wp_b = constp.tile([64, DIM], BF16)
                nc.sync.dma_start(wp_a[:], wproj_d[0:128, :])
                nc.sync.dma_start(wp_b[:], wproj_d[128:192, :])
                ident = constp.tile([128, 128], F32)
                nc.sync.dma_start(ident[:], ident_d[:])
                ident_bf = constp.tile([128, 128], BF16)
                nc.scalar.copy(ident_bf[:], ident[:])

                def w8u(t, unit):
                    base = (t * 12 + unit) * 2 * 128
                    return wdw8[:, base : base + 2 * 128].rearrange(
                        "c (b m) -> c b m", b=2)

                def wvdiag(tv, kw):
                    base = (tv * 3 + kw) * 128
                    nt = V_T[tv][1]
                    return wdwv[0:nt, base : base + nt]

                def wvdiag2(tv, kw):
                    base = (6 + tv * 3 + kw) * 128
                    nt = V_T[tv][1]
                    return wdwv[0:nt, base : base + nt]

                def wvcol(tv, idx, nt):
                    return wvs[0:nt, 7 * tv + idx : 7 * tv + idx + 1]

                def bdwq_col(t):
                    return wvs[:, 14 + t : 15 + t]

                ring8f = [constp.tile([128, RING * W + 512], FP8,
                                      name=f"ring8_{t}") for t in range(3)]
                ring8 = [r[:, 0 : RING * W].rearrange("c (r w) -> c r w",
                                                      r=RING) for r in ring8f]
                ringv = [constp.tile([128, RINGV, W], BF16, name="ringv0"),
                         constp.tile([64, RINGV, W], BF16, name="ringv1")]
                v0 = constp.tile([128, HALF, W], BF16)
                v1 = constp.tile([64, HALF, W], BF16)

                g_bin = dram.tile([48, 200], F32)
                g_bout = dram.tile([48, 200], F32)

                with (
                    tc.tile_pool(name="qkps", bufs=2, space="PSUM") as qkps,
                    tc.tile_pool(name="dwps", bufs=2, space="PSUM") as dwps,
                    tc.tile_pool(name="tps", bufs=2, space="PSUM") as tps,
                    tc.tile_pool(name="gram", bufs=1, space="PSUM") as gramp,
                    tc.tile_pool(name="selfg", bufs=1, space="PSUM") as selfp,
                ):
                    gram_ps = gramp.tile([48, 4 * 48], F32)
                    self_ps = selfp.tile([48, 2 * 192], F32)
                    first_g = [True]

                    for j in range(NITER):
                        rj8 = (2 * j) % RING
                        rjv = (2 * j) % RINGV
                        # ---- load x, qkv conv, ring copies ----
                        xa8 = xpool.tile([97, 2, 2, W], FP8, tag="x8")
                        nc.sync.dma_start(
                            xa8[:], x8_d[:, :, 2 * j : 2 * j + 2, :])
                        xv0 = xpool.tile([128, 2, W], BF16, tag="xv0")
                        xv1 = xpool.tile([65, 2, W], BF16, tag="xv1")
                        nc.sync.dma_start(
                            xv0[:], xbf_d[0:128, 2 * j : 2 * j + 2, :])
                        nc.sync.dma_start(
                            xv1[:], xbf_d[128:193, 2 * j : 2 * j + 2, :])
                        for t, (c0, nt) in enumerate(QK_T):
                            ps = qkps.tile([128, 2, W], F32, tag="qkps")
                            nc.tensor.matmul(
                                ps[0:nt], wq8[:, :, c0 : c0 + nt],
                                xa8.rearrange("p b r w -> p b (r w)"),
                                start=True, stop=True, perf_mode=DR)
                            nc.scalar.copy(
                                ring8[t][0:nt, rj8 : rj8 + 2, :], ps[0:nt])
                        for tv, (c0, nt) in enumerate(V_T):
                            ps = qkps.tile([128, 2, W], F32, tag="qkps")
                            nc.tensor.matmul(ps[0:nt], wqv_a[:, c0 : c0 + nt],
                                             xv0[:], start=True, stop=False)
                            nc.tensor.matmul(ps[0:nt], wqv_b[:, c0 : c0 + nt],
                                             xv1[:], start=False, stop=True)
                            nc.scalar.copy(
                                ringv[tv][0:nt, rjv : rjv + 2, :], ps[0:nt])

                        if j < 1:
                            continue
                        i = j - 1  # output pair
                        r0 = (2 * i) % RING
                        r1 = (2 * i + 1) % RING
                        r2 = (2 * i + 2) % RING
                        wrap8 = r0 == RING - 2
                        v0r = (2 * i) % RINGV
                        v1r = (2 * i + 1) % RINGV
                        v2r = (2 * i + 2) % RINGV
                        wrapv = v1r == RINGV - 1

                        # ---- dw for q,k tiles: all-PE fp8, single-row DR ----
                        dq = []
                        for t, (c0, nt) in enumerate(QK_T):
                            dps = dwps.tile([128, 2, W], F32, tag="dwps")
                            flat = ring8f[t]
                            first = True
                            for row in range(2):
                                pa = (2 * i + row) % RING       # kh0 in-row
                                pc = (2 * i + 2 + row) % RING   # kh2 in-row
                                pb = (2 * i + 1 + row) % RING   # kh1 in-row
                                for kw in (1, 0, 2):
                                    iw, ow = _colwin(kw)
                                    ncol = iw.stop - iw.start
                                    if pc > pa:
                                        base = pa * W + iw.start
                                        ifm = flat[0:nt, base : base + 1024] \
                                            .rearrange("c (b z) -> c b z", b=2)[
                                                :, :, 0:ncol]
                                        nc.tensor.matmul(
                                            dps[0:nt, row, ow],
                                            w8u(t, kw)[0:nt], ifm,
                                            start=first, stop=False,
                                            perf_mode=DR)
                                        first = False
                                    else:  # ring wrap: two one-tap dummies
                                        for ubase, rr in ((6, pa), (9, pc)):
                                            win = ring8[t][0:nt, rr, iw]
                                            ifm = win.unsqueeze(1) \
                                                .broadcast_to([nt, 2, ncol])
                                            nc.tensor.matmul(
                                                dps[0:nt, row, ow],
                                                w8u(t, ubase + kw)[0:nt], ifm,
                                                start=first, stop=False,
                                                perf_mode=DR)
                                            first = False
                                    # kh1 dummy-DR
                                    win = ring8[t][0:nt, pb, iw]
                                    ifm = win.unsqueeze(1).broadcast_to(
                                        [nt, 2, ncol])
                                    nc.tensor.matmul(
                                        dps[0:nt, row, ow],
                                        w8u(t, 3 + kw)[0:nt], ifm,
                                        start=False,
                                        stop=(row == 1 and kw == 2),
                                        perf_mode=DR)
                            d = dqpool.tile([128, 2, W], BF16, tag=f"dq{t}")
                            nc.scalar.activation(d[0:nt], dps[0:nt],
                                                 AF.Identity, bias=bdwq_col(t))
                            dq.append(d)

                        # ---- dw for v tiles: PE kh1 + chains ----
                        for tv, (c0, nt) in enumerate(V_T):
                            rv = ringv[tv]
                            dps = dwps.tile([128, 2, W], F32, tag="dwps")
                            for kh, kw, first, last in (
                                (1, 1, True, False), (1, 0, False, False),
                                (1, 2, False, True),
                            ):
                                iw, ow = _colwin(kw)
                                wv = (wvdiag(tv, kw) if kh == 1
                                      else wvdiag2(tv, kw))
                                if kw == 1 and (2 * i + kh) % RINGV != RINGV - 1:
                                    p0 = (2 * i + kh) % RINGV
                                    nc.tensor.matmul(
                                        dps[0:nt, :, ow], wv,
                                        rv[0:nt, p0 : p0 + 2, iw],
                                        start=first, stop=last)
                                else:
                                    for row in range(2):
                                        rr = (2 * i + kh + row) % RINGV
                                        nc.tensor.matmul(
                                            dps[0:nt, row, ow], wv,
                                            rv[0:nt, rr, iw],
                                            start=first and row == 0,
                                            stop=last and row == 1)
                            acc = accp.tile([128, 2, W], BF16, tag=f"acc{tv}")
                            # init: kh0 kw1 full width, + b_dw (DVE 2-scalar ts)
                            nc.vector.tensor_scalar(
                                acc[0:nt], rv[0:nt, v0r : v0r + 2, :],
                                wvcol(tv, 1, nt), wvcol(tv, 6, nt),
                                op0=OP.mult, op1=OP.add)
                            # conts: product on DVE/Pool ts, add on DVE TT
                            for eng, rr, idx, kw in (
                                ("p", v0r, 0, 0), ("p", v0r, 2, 2),
                                ("p", v2r, 3, 0), ("pp", v2r, 4, 1),
                                ("p", v2r, 5, 2),
                            ):
                                iw, ow = _colwin(kw)
                                ncol = iw.stop - iw.start
                                tmp = accp.tile([128, 2, W], BF16,
                                                tag=f"tmp{tv}")
                                e = nc.gpsimd if eng.startswith("p") else nc.vector
                                e.tensor_scalar_mul(
                                    tmp[0:nt, :, 0:ncol],
                                    rv[0:nt, rr : rr + 2, iw],
                                    wvcol(tv, idx, nt))
                                e2 = nc.gpsimd if eng == "pp" else nc.vector
                                e2.tensor_tensor(
                                    acc[0:nt, :, ow], acc[0:nt, :, ow],
                                    tmp[0:nt, :, 0:ncol], op=OP.add)
                            vdst = (v0 if tv == 0 else v1)[
                                0:nt, 2 * i : 2 * i + 2, :]
                            e = nc.vector
                            e.scalar_tensor_tensor(
                                vdst, dps[0:nt], 1.0, acc[0:nt],
                                op0=OP.mult, op1=OP.add)

                        # ---- transposes + gram + selfgram ----
                        for sp in range(2):
                            tp = tps.tile([128, 2, 384], BF16, tag="tps")
                            for sc in range(2):
                                for t in range(3):
                                    chunk = dq[t].rearrange(
                                        "c r w -> c (r w)")[
                                        :, 128 * (2 * sp + sc) :
                                        128 * (2 * sp + sc) + 128]
                                    nc.tensor.transpose(
                                        tp[:, sc, 128 * t : 128 * t + 128],
                                        chunk, ident_bf[:])
                            sT2 = sTpool.tile([128, 2, 384], BF16, tag="sT")
                            nc.vector.tensor_copy(sT2[:], tp[:])
                            for sc in range(2):
                              sT = sT2[:, sc]
                              for h in range(NH):
                                qs = sT[:, 48 * h : 48 * h + 48]
                                ks = sT[:, 192 + 48 * h : 240 + 48 * h]
                                last = (i == NDW - 1 and sp == 1 and sc == 1
                                        and h == NH - 1)
                                st = first_g[0]
                                nc.tensor.matmul(
                                    gram_ps[:, 48 * h : 48 * h + 48], qs, ks,
                                    start=st, stop=last, skip_group_check=True)
                                nc.tensor.matmul(
                                    self_ps[:, 48 * h : 48 * h + 48], qs, qs,
                                    start=st, stop=last, skip_group_check=True)
                                nc.tensor.matmul(
                                    self_ps[:, 192 + 48 * h : 240 + 48 * h],
                                    ks, ks,
                                    start=st, stop=last, skip_group_check=True)
                                first_g[0] = False

                    # ---- phase B ----
                    cc = smallp.tile([48, 200], F32)
                    nc.vector.tensor_copy(cc[:, 0:192], gram_ps[:])
                    self_sb = smallp.tile([48, 384], F32)
                    nc.vector.tensor_copy(self_sb[:], self_ps[:])
                    dtmp = smallp.tile([48, 48], F32)
                    for h in range(8):
                        nc.vector.tensor_tensor(
                            dtmp[:], self_sb[:, 48 * h : 48 * h + 48],
                            ident[0:48, 0:48], op=OP.mult)
                        nc.vector.tensor_reduce(
                            cc[:, 192 + h : 193 + h], dtmp[:], AX.X, OP.add)
                    nc.sync.dma_start(g_bin[:], cc[:])
                    if no_cc:
                        nc.sync.dma_start(g_bout[:], g_bin[:])
                    else:
                        groups = [[0, 1], [2, 3], [4, 5], [6, 7]]
                        nc.gpsimd.collective_compute(
                            "AllReduce", OP.add, replica_groups=groups,
                            ins=[g_bin[:]], outs=[g_bout[:]])
                    g2 = smallp.tile([48, 200], F32)
                    nc.sync.dma_start(g2[:], g_bout[:])

                    nrm = smallp.tile([48, 8], F32)
                    nc.scalar.sqrt(nrm[:], g2[:, 192:200])
                    nc.vector.tensor_scalar_max(nrm[:], nrm[:], EPS)
                    rn = smallp.tile([48, 8], F32)
                    nc.vector.reciprocal(rn[:], nrm[:])

                    att = smallp.tile([48, 192], F32)
                    mxs = smallp.tile([48, NH], F32)
                    sm = smallp.tile([48, NH], F32)
                    rs = smallp.tile([48, NH], F32)
                    for h in range(NH):
                        sl = slice(48 * h, 48 * h + 48)
                        nc.vector.tensor_scalar_mul(g2[:, sl], g2[:, sl],
                                                    rn[:, h : h + 1])
                        tp = dwps.tile([128, 128], F32, tag="dwps")
                        nc.tensor.transpose(tp[0:48, 0:48], g2[:, sl],
                                            ident[0:48, 0:48])
                        gt = smallp.tile([48, 48], F32, tag="gt")
                        nc.scalar.activation(gt[:], tp[0:48, 0:48], AF.Identity,
                                             scale=rn[:, 4 + h : 5 + h])
                        tp2 = dwps.tile([128, 128], F32, tag="dwps")
                        nc.tensor.transpose(tp2[0:48, 0:48], gt[:],
                                            ident[0:48, 0:48])
                        nc.vector.tensor_copy(g2[:, sl], tp2[0:48, 0:48])
                        nc.vector.tensor_reduce(mxs[:, h : h + 1], g2[:, sl],
                                                AX.X, OP.max, negate=True)
                        nc.vector.tensor_scalar_mul(mxs[:, h : h + 1],
                                                    mxs[:, h : h + 1], SCALE)
                        nc.scalar.activation(att[:, sl], g2[:, sl], AF.Exp,
                                             bias=mxs[:, h : h + 1], scale=SCALE)
                        nc.vector.tensor_reduce(sm[:, h : h + 1], att[:, sl],
                                                AX.X, OP.add)
                        nc.vector.reciprocal(rs[:, h : h + 1], sm[:, h : h + 1])
                        nc.vector.tensor_scalar_mul(att[:, sl], att[:, sl],
                                                    rs[:, h : h + 1])
                    att_bf = smallp.tile([48, 192], BF16)
                    nc.vector.tensor_copy(att_bf[:], att[:])
                    abd0 = smallp.tile([128, 192], BF16)
                    abd1 = smallp.tile([64, 192], BF16)
                    nc.gpsimd.memset(abd0[:], 0.0)
                    nc.gpsimd.memset(abd1[:], 0.0)
                    nc.vector.tensor_copy(abd0[0:48, 0:48], att_bf[:, 0:48])
                    nc.sync.dma_start(abd0[48:96, 48:96], att_bf[:, 48:96])
                    nc.sync.dma_start(abd0[96:128, 96:144],
                                      att_bf[0:32, 96:144])
                    nc.sync.dma_start(abd1[0:16, 96:144], att_bf[32:48, 96:144])
                    nc.sync.dma_start(abd1[16:64, 144:192], att_bf[:, 144:192])
                    # PT[e, o] = sum_c A_bd[c, e] * wprojT[c, o]
                    pt_ps = dwps.tile([128, 192], F32, tag="dwps")
                    nc.tensor.matmul(pt_ps[:], abd0[:, 0:128], wp_a[:],
                                     start=True, stop=False)
                    nc.tensor.matmul(pt_ps[:], abd1[:, 0:128], wp_b[:],
                                     start=False, stop=True)
                    pta = smallp.tile([128, 192], BF16)
                    nc.vector.tensor_copy(pta[:], pt_ps[:])
                    pt_ps2 = dwps.tile([128, 192], F32, tag="dwps")
                    nc.tensor.matmul(pt_ps2[0:64], abd0[:, 128:192], wp_a[:],
                                     start=True, stop=False)
                    nc.tensor.matmul(pt_ps2[0:64], abd1[:, 128:192], wp_b[:],
                                     start=False, stop=True)
                    ptb = smallp.tile([64, 192], BF16)
                    nc.vector.tensor_copy(ptb[:], pt_ps2[0:64])
                    if dbg:
                        nc.sync.dma_start(dbg1_d[:, 0:200], cc[:])
                        nc.sync.dma_start(dbg1_d[:, 200:400], g2[:])
                        nc.sync.dma_start(dbg1_d[:, 400:592], att[:])
                        dbf = smallp.tile([128, 384], F32)
                        nc.gpsimd.memset(dbf[:], 0.0)
                        nc.vector.tensor_copy(dbf[:, 0:192], pta[:])
                        nc.vector.tensor_copy(dbf[0:64, 192:384], ptb[:])
                        nc.sync.dma_start(dbg2_d[:], dbf[:])

                # ---- phase C: out = PT.T @ v + b_proj ----
                with (
                    tc.tile_pool(name="pcps", bufs=2, space="PSUM") as pcps,
                ):
                    for i in range(NDW):
                        vs0 = v0[:, 2 * i : 2 * i + 2, :]
                        vs1 = v1[:, 2 * i : 2 * i + 2, :]
                        pp0 = pcps.tile([128, 2, W], F32, tag="pc0")
                        nc.tensor.matmul(pp0[:], pta[:, 0:128], vs0,
                                         start=True, stop=False)
                        nc.tensor.matmul(pp0[:], ptb[:, 0:128], vs1,
                                         start=False, stop=True)
                        pp1 = pcps.tile([128, 2, W], F32, tag="pc1")
                        nc.tensor.matmul(pp1[0:64], pta[:, 128:192], vs0,
                                         start=True, stop=False)
                        nc.tensor.matmul(pp1[0:64], ptb[:, 128:192], vs1,
                                         start=False, stop=True)
                        ob0 = outsbp.tile([128, 2, W], F32, tag="ob0")
                        ob1 = outsbp.tile([64, 2, W], F32, tag="ob1")
                        nc.scalar.activation(ob0[:], pp0[:], AF.Identity,
                                             bias=wvs[:, 17:18])
                        nc.scalar.activation(ob1[:], pp1[0:64], AF.Identity,
                                             bias=wvs[0:64, 18:19])
                        nc.sync.dma_start(
                            out_d[0:128, 2 * i : 2 * i + 2, :], ob0[:])
                        nc.sync.dma_start(
                            out_d[128:192, 2 * i : 2 * i + 2, :], ob1[:])

    nc.compile()
    return nc


def _get_nc(repeat=1, no_cc=False, dbg=False):
    key = (repeat, no_cc, dbg)
    if key not in _CACHED:
        _CACHED[key] = _build_nc(repeat, no_cc, dbg)
    return _CACHED[key]


FP8NP = ml_dtypes.float8_e4m3fn
BF16NP = ml_dtypes.bfloat16


def _prep_inputs(x, w_qkv, b_qkv, w_dw, b_dw, w_proj, b_proj):
    x = np.asarray(x, np.float32)
    wq = np.asarray(w_qkv, np.float32)[:, :, 0, 0]        # [576, 192]
    bq = np.asarray(b_qkv, np.float32)
    wd = np.asarray(w_dw, np.float32)[:, 0]               # [576, 3, 3]
    bd = np.asarray(b_dw, np.float32)
    wp = np.asarray(w_proj, np.float32)[:, :, 0, 0]       # [192, 192]
    bp = np.asarray(b_proj, np.float32)

    wqkvT = wq.T                                          # [192, 576]
    ident = np.eye(128, dtype=np.float32)
    xp = np.pad(x, ((0, 0), (0, 0), (1, 1), (0, 0)))      # [4, 192, 258, 256]

    wq8 = np.zeros((97, 2, 384), np.float32)
    for i in range(2):
        wq8[0:96, i, :] = wqkvT[96 * i : 96 * i + 96, 0:384]
    wq8[96, 0, :] = bq[0:384]
    wqv = np.zeros((193, 192), np.float32)
    wqv[0:192] = wqkvT[:, 384:576]
    wqv[192] = bq[384:576]

    wdw8 = np.zeros((128, 3 * 12 * 2 * 128), np.float32)
    idx = np.arange(128)
    for t in range(3):
        ch = 128 * t + idx

        def setd(unit, blk, kh, kw):
            base = (t * 12 + unit) * 2 * 128 + blk * 128
            wdw8[idx, base + idx] = wd[ch, kh, kw]

        for kw in range(3):
            setd(kw, 0, 0, kw)       # pair block0 = kh0
            setd(kw, 1, 2, kw)       # pair block1 = kh2
            setd(3 + kw, 0, 1, kw)   # kh1 dummy (block1 stays 0)
            setd(6 + kw, 0, 0, kw)   # kh0-only dummy (wrap iters)
            setd(9 + kw, 0, 2, kw)   # kh2-only dummy (wrap iters)

    wdwv = np.zeros((128, 2 * 2 * 3 * 128), np.float32)
    wvs = np.zeros((128, 19), np.float32)
    for tv, (c0, nt) in enumerate(V_T):
        ch = 384 + c0 + np.arange(nt)
        for kw in range(3):
            base = (tv * 3 + kw) * 128
            wdwv[np.arange(nt), base + np.arange(nt)] = wd[ch, 1, kw]
            base2 = (6 + tv * 3 + kw) * 128
            wdwv[np.arange(nt), base2 + np.arange(nt)] = wd[ch, 2, kw]
        for k, (kh, kw) in enumerate(((0, 0), (0, 1), (0, 2),
                                      (2, 0), (2, 1), (2, 2))):
            wvs[0:nt, 7 * tv + k] = wd[ch, kh, kw]
        wvs[0:nt, 7 * tv + 6] = bd[ch]
    for t in range(3):
        wvs[:, 14 + t] = bd[128 * t : 128 * t + 128]
    wvs[:, 17] = bp[0:128]
    wvs[0:64, 18] = bp[128:192]

    wprojT = np.ascontiguousarray(wp.T)
    in_maps = []
    for core in range(N_CORES):
        b, hf = divmod(core, 2)
        x_sh = xp[b, :, hf * HALF : hf * HALF + PR, :]    # [192, 130, 256]
        ones = np.ones((PR, W), np.float32)
        if hf == 0:
            ones[0] = 0.0
        else:
            ones[PR - 1] = 0.0
        x8 = np.zeros((97, 2, PR, W), np.float32)
        for i in range(2):
            x8[0:96, i] = x_sh[96 * i : 96 * i + 96]
        x8[96, 0] = ones
        xbf = np.concatenate([x_sh, ones[None]], axis=0)  # [193, 130, 256]
        in_maps.append({
            "x8": x8.astype(FP8NP), "xbf": xbf.astype(BF16NP),
            "wq8": wq8.astype(FP8NP), "wqv": wqv.astype(BF16NP),
            "wdw8": wdw8.astype(FP8NP), "wdwv": wdwv.astype(BF16NP),
            "wvs": np.ascontiguousarray(wvs),
            "wprojT": wprojT.astype(BF16NP), "ident": ident,
        })
    return in_maps


def kernel(x, w_qkv, b_qkv, w_dw, b_dw, w_proj, b_proj):
    nc = _get_nc()
    in_maps = _prep_inputs(x, w_qkv, b_qkv, w_dw, b_dw, w_proj, b_proj)
    res = run_bass_kernel_spmd(nc, in_maps, core_ids=list(range(N_CORES)))
    out = np.empty((B, DIM, H, W), np.float32)
    for core in range(N_CORES):
        b, hf = divmod(core, 2)
        out[b, :, hf * HALF : (hf + 1) * HALF, :] = res.results[core]["out_sh"]
    return out
